# revision 12
# baseline (speedup 1.0000x reference)
"""Trainium2 Bass kernel for nn_CustomizedSelfAttention.

Reference computation (per batch sample b):
    q = x @ Wq; k = x @ Wk; v = x @ Wv
    attn = softmax(q @ k.T * C**-0.5)          # [N, N]
    y = attn @ v @ Wp + bp + x                 # [N, C]
    pooled = mean(y, axis=0)                   # [C]
    out = relu(pooled @ Wf1 + bf1) @ Wf2 + bf2 # [C]

Key algebraic collapse (exact): only the token-mean of the attention output
is needed, so with  t[m] = sum_n softmax_row_n[m]  (column sums of the
attention matrix):
    mean_n(attn @ v) = (t/N) @ v = ((t/N) @ x) @ Wv
    pooled = ((t/N) @ x) @ (Wv @ Wp) + bp + mean_n(x)
This removes the O(N^2 C) attn@v matmul and the O(N C^2) v/p projections.
Also  q @ k.T * s = x @ (Wq @ Wk.T * s) @ x.T = x @ A @ x.T  with A fused on
the host, removing one more projection.

Per-core device work (v2 layout):
  phase 0 (per 1024-token chunk, pipelined with DMA):
    xb  = x in bf16, resident in SBUF (DMA casts f32->bf16)
    xT  = PE transpose of 128x128 blocks, packed 7-per-PSUM-bank, cast to
          fp8 feature-major layout [P, 4, 2, N] via wide ACT/DVE copies
    GT  = A.T-side projection for the chunk (fp8 DoubleRow matmuls),
          gt_full [P, 4, 2, N] fp8 kept resident
  S loop (per 128-token row block nt):
    S   = gt.T @ xt per 512-col chunk pairs (fp8 DR, PSUM f32)
    E   = exp(S/ASC) -> SBUF bf16 + row sums Z via ACT accum_out
    esum accumulation: E scaled by 1/Z on DVE, grouped by 8 row blocks
    tT  += column sums of esum via PE matmuls (ones vector moving)
  tail (bf16 PE matmuls):
    Y2[2, C] = sum_j [t/N | 1/N]_j.T @ xb_j  -> u = (t/N)@x, xbar = mean(x)
    pooled = u @ Wvp + bp + xbar ; h = relu(pooled @ Wf1 + bf1)
    out = h @ Wf2 + bf2   (128x128-block mat-vec chains, bf16 weights)

Sharding: data-parallel over batch, 1 sample per core, weights replicated.
"""

import numpy as np
import ml_dtypes
from contextlib import ExitStack

import concourse.bass as bass
import concourse.tile as tile
from concourse import bacc, mybir
from concourse.bass_utils import run_bass_kernel_spmd

B, N, C = 8, 4096, 896
NCORES = 8
P = 128
CCH = C // P          # 7 feature chunks of 128
NT = N // P           # 32 token tiles of 128
MCH = 512             # S free-dim chunk
NMC = N // MCH        # 8
TCH = 512             # token half-chunk for G
TCHM = 2 * TCH        # 1024-token chunk (fp8 DR)
NCHG = N // TCHM      # 4 chunks
BF16 = mybir.dt.bfloat16
FP8 = mybir.dt.float8e4
F32 = mybir.dt.float32

_BF = ml_dtypes.bfloat16
_F8 = ml_dtypes.float8_e4m3

ASC = 128.0        # fp8 scale folded into A, undone in exp's affine
CG8 = 4            # fp8 c-groups of 256 (C padded 896 -> 1024)
GRP = 16           # n-tiles accumulated (r-scaled) before colsum
NGRP = NT // GRP


def _build_body(ctx: ExitStack, tc: "tile.TileContext", aps: dict):
    nc = tc.nc
    x_d = aps["xc"]
    a_d = aps["abf"]
    w_ds = (aps["wvp"], aps["wf1"], aps["wf2"])
    bias_d = aps["biasR"]
    ident_d = aps["ident"]
    out_d = aps["outT"]

    const_pool = ctx.enter_context(tc.tile_pool(name="const", bufs=1))
    a_pool = ctx.enter_context(tc.tile_pool(name="a", bufs=1))
    xb_pool = ctx.enter_context(tc.tile_pool(name="xb", bufs=1))
    xt_pool = ctx.enter_context(tc.tile_pool(name="xt", bufs=1))
    gt_pool = ctx.enter_context(tc.tile_pool(name="gt", bufs=1))
    e_pool = ctx.enter_context(tc.tile_pool(name="e", bufs=3))
    small_pool = ctx.enter_context(tc.tile_pool(name="small", bufs=4))
    w_pool = ctx.enter_context(tc.tile_pool(name="w", bufs=1))
    tail_pool = ctx.enter_context(tc.tile_pool(name="tail", bufs=1))
    ps2_pool = ctx.enter_context(tc.tile_pool(name="ps2", bufs=2, space="PSUM"))
    tp_pool = ctx.enter_context(tc.tile_pool(name="tp", bufs=2, space="PSUM"))
    acc_pool = ctx.enter_context(tc.tile_pool(name="acc", bufs=1, space="PSUM"))

    # --- resident x (bf16) : DMA casts f32->bf16 (gpsimd-only feature) ---
    xb = xb_pool.tile([P, NT, C], BF16, tag="xb")
    for j in range(NT):
        nc.gpsimd.dma_start(xb[:, j, :], x_d[j * P:(j + 1) * P, :])

    # --- constants / weights (sync queue, overlaps with x DMA) ---
    ident = const_pool.tile([P, P], BF16, tag="ident")
    nc.sync.dma_start(ident[:], ident_d)
    bias_sb = const_pool.tile([P, 3 * CCH], F32, tag="bias")
    nc.sync.dma_start(bias_sb[:], bias_d)
    ones1 = const_pool.tile([P, 1], BF16, tag="ones1")
    nc.vector.memset(ones1[:], 1.0)
    # A pre-scaled by ASC, zero-padded to 1024 rows on host.
    a_sb = a_pool.tile([P, CG8, 2, C], FP8, tag="a")
    for bb in range(2 * CG8):
        nc.sync.dma_start(a_sb[:, bb // 2, bb % 2, :],
                          a_d[bb * P:(bb + 1) * P, :])
    # tail weights (bf16 on host)
    wS = w_pool.tile([P, 3, CCH, C], BF16, tag="w")
    for i, w_d in enumerate(w_ds):
        nc.sync.dma_start(wS[:, i, :, :],
                          w_d.rearrange("(cc p) e -> p cc e", p=P))

    # --- phase 0: transpose x into fp8 xt + G projection, chunk-pipelined ---
    # pad slots [CG8-1, 1] are never read (the 4th feature group runs as a
    # plain fp8 matmul over the 128 real features), so no memsets needed
    xt = xt_pool.tile([P, CG8, 2, N], FP8, tag="xt")
    gt = gt_pool.tile([P, CG8, 2, N], FP8, tag="gt")

    def emit_transpose_tile(ch, jj):
        j = ch * (TCHM // P) + jj
        tpt = tp_pool.tile([P, CCH, P], BF16, tag="tp")
        for cc in range(CCH):
            nc.tensor.transpose(tpt[:, cc, :],
                                xb[:, j, cc * P:(cc + 1) * P], ident[:])
        sl = slice(j * P, (j + 1) * P)
        # cast bf16->fp8 into the DR-layout slots; split ACT/DVE
        nc.scalar.copy(xt[:, 0:2, :, sl], tpt[:, 0:4, :])
        nc.vector.tensor_copy(xt[:, 2, :, sl], tpt[:, 4:6, :])
        nc.vector.tensor_copy(xt[:, 3, 0, sl], tpt[:, 6, :])

    DR = mybir.MatmulPerfMode.DoubleRow

    def emit_g_dd(ch, dd):
        # features 0..767 via 3 DoubleRow groups; 768..895 via one plain
        # fp8 matmul (the DR pad half 896..1023 would be all zeros)
        gp = ps2_pool.tile([P, 2, TCH], F32, tag="ps", name="gp")
        for g in range(CG8 - 1):
            for h in range(2):
                nc.tensor.matmul(
                    gp[:, h, :], a_sb[:, g, :, dd * P:(dd + 1) * P],
                    xt[:, g, :,
                       ch * TCHM + h * TCH:ch * TCHM + (h + 1) * TCH],
                    start=(g == 0), stop=False,
                    perf_mode=DR, skip_group_check=True,
                )
        for h in range(2):
            nc.tensor.matmul(
                gp[:, h, :], a_sb[:, CG8 - 1, 0, dd * P:(dd + 1) * P],
                xt[:, CG8 - 1, 0,
                   ch * TCHM + h * TCH:ch * TCHM + (h + 1) * TCH],
                start=False, stop=True, skip_group_check=True,
            )
        nc.vector.tensor_copy(gt[:, dd // 2, dd % 2,
                                 ch * TCHM:(ch + 1) * TCHM], gp[:])

    # Interleave G(ch-1) dd-groups between transpose tiles of chunk ch so
    # the PE sees matmul activity every ~2us (transposes alone don't keep
    # the HAM clock-gate warm).
    for jj in range(TCHM // P):
        emit_transpose_tile(0, jj)
    for ch in range(1, NCHG):
        for jj in range(TCHM // P):
            emit_transpose_tile(ch, jj)
            if jj < CCH:
                emit_g_dd(ch - 1, jj)
    for dd in range(CCH):
        emit_g_dd(NCHG - 1, dd)

    # --- S loop ---
    tT = acc_pool.tile([P, NT], F32, tag="acc")  # column-sum accumulator
    pend_colsum = []           # deferred (group_idx, esum) colsum emissions

    def emit_colsum(gidx, es):
        # Single accumulation group over the whole kernel: start only on the
        # very first matmul into the bank, stop only on the very last.
        for j in range(NT):
            nc.tensor.matmul(
                tT[:, j:j + 1], es[:, j * P:(j + 1) * P], ones1[:],
                start=(gidx == 0 and j == 0),
                stop=(gidx == NGRP - 1 and j == NT - 1),
                skip_group_check=True,
            )

    esum = None
    for nt in range(NT):
        while pend_colsum:
            emit_colsum(*pend_colsum.pop(0))
        e_t = e_pool.tile([P, N], BF16, tag="e")
        zp = small_pool.tile([P, NMC // 2], F32, tag="zp")
        for mj2 in range(NMC // 2):
            sps = ps2_pool.tile([P, 2, MCH], F32, tag="ps", name="sps")
            for g in range(CG8 - 1):
                for k in range(2):
                    mj = 2 * mj2 + k
                    nc.tensor.matmul(
                        sps[:, k, :], gt[:, g, :, nt * P:(nt + 1) * P],
                        xt[:, g, :, mj * MCH:(mj + 1) * MCH],
                        start=(g == 0), stop=False,
                        perf_mode=DR, skip_group_check=True,
                    )
            for k in range(2):
                mj = 2 * mj2 + k
                nc.tensor.matmul(
                    sps[:, k, :], gt[:, CG8 - 1, 0, nt * P:(nt + 1) * P],
                    xt[:, CG8 - 1, 0, mj * MCH:(mj + 1) * MCH],
                    start=False, stop=True, skip_group_check=True,
                )
            nc.scalar.activation(
                e_t[:, mj2 * 2 * MCH:(mj2 + 1) * 2 * MCH], sps[:],
                mybir.ActivationFunctionType.Exp,
                scale=1.0 / ASC,
                accum_out=zp[:, mj2:mj2 + 1],
            )

        z = small_pool.tile([P, 1], F32, tag="z")
        nc.vector.reduce_sum(z[:], zp[:], axis=mybir.AxisListType.X)
        rf = small_pool.tile([P, 1], F32, tag="rf")
        nc.vector.reciprocal(rf[:], z[:])
        # fold 1/Z into E so the colsum needs no per-tile weights
        gi, gq = nt // GRP, nt % GRP
        if gq == 0:
            # bufs=2: esum(g) is drained by colsum while esum(g+1) accumulates
            esum = e_pool.tile([P, N], BF16, tag="esum", bufs=2)
            nc.vector.tensor_scalar_mul(esum[:], e_t[:], rf[:])
        else:
            nc.vector.tensor_scalar_mul(e_t[:], e_t[:], rf[:])
            nc.vector.tensor_add(esum[:], esum[:], e_t[:])
        if gq == GRP - 1:
            pend_colsum.append((gi, esum))
    while pend_colsum:
        emit_colsum(*pend_colsum.pop(0))

    # --- tail (bf16 matmuls against resident xb) ---
    # TO[:, j, 0] = t[j-th chunk]/N ; TO[:, j, 1] = 1/N
    TO = tail_pool.tile([P, NT, 2], BF16, tag="to")
    nc.vector.memset(TO[:, :, 1], 1.0 / N)
    nc.scalar.mul(TO[:, :, 0], tT[:], 1.0 / N)

    # Y2[row, c]: row 0 = u = (t/N)@x, row 1 = xbar (mean of x over tokens)
    Y2 = ps2_pool.tile([2, C], F32, tag="ps", name="y2")
    for j in range(NT):
        for (o, w) in ((0, 512), (512, 384)):
            nc.tensor.matmul(
                Y2[:, o:o + w], TO[:, j, :], xb[:, j, o:o + w],
                start=(j == 0), stop=(j == NT - 1),
                skip_group_check=True,
            )
    y2S = tail_pool.tile([2, C], BF16, tag="y2s")
    nc.scalar.copy(y2S[:], Y2[:])
    identF2 = tail_pool.tile([2, 2], BF16, tag="idf2")
    nc.vector.tensor_copy(identF2[:], ident[0:2, 0:2])
    uB = tail_pool.tile([P, CCH], BF16, tag="ub")       # u, bf16 (matvec in)
    xbarS = tail_pool.tile([P, CCH], F32, tag="xbar")   # xbar, f32
    for cc in range(CCH):
        ptx = ps2_pool.tile([P, 2], BF16, tag="ps", name="ptx")
        nc.tensor.transpose(ptx[:], y2S[:, cc * P:(cc + 1) * P], identF2[:])
        nc.vector.tensor_copy(uB[:, cc:cc + 1], ptx[:, 0:1])
        nc.vector.tensor_copy(xbarS[:, cc:cc + 1], ptx[:, 1:2])

    def matvec(wi, vec, out_psum):
        # out_psum[:, ee] = sum_cc W[cc-block, ee-block].T @ vec[:, cc]
        for ee in range(CCH):
            for cc in range(CCH):
                nc.tensor.matmul(
                    out_psum[:, ee:ee + 1],
                    wS[:, wi, cc, ee * P:(ee + 1) * P],
                    vec[:, cc:cc + 1],
                    start=(cc == 0), stop=(cc == CCH - 1),
                )

    P2 = ps2_pool.tile([P, CCH], F32, tag="ps", name="p2")
    matvec(0, uB, P2)
    pooledS = tail_pool.tile([P, CCH], F32, tag="pooled")
    nc.vector.tensor_add(pooledS[:], P2[:], xbarS[:])
    nc.vector.tensor_add(pooledS[:], pooledS[:], bias_sb[:, 0:CCH])
    pooledB = tail_pool.tile([P, CCH], BF16, tag="pooledb")
    nc.vector.tensor_copy(pooledB[:], pooledS[:])

    H2 = ps2_pool.tile([P, CCH], F32, tag="ps", name="h2")
    matvec(1, pooledB, H2)
    hS = tail_pool.tile([P, CCH], F32, tag="h")
    nc.vector.tensor_add(hS[:], H2[:], bias_sb[:, CCH:2 * CCH])
    nc.vector.tensor_scalar_max(hS[:], hS[:], 0.0)
    hB = tail_pool.tile([P, CCH], BF16, tag="hb")
    nc.vector.tensor_copy(hB[:], hS[:])

    O2 = ps2_pool.tile([P, CCH], F32, tag="ps", name="o2")
    matvec(2, hB, O2)
    outS = tail_pool.tile([P, CCH], F32, tag="out")
    nc.vector.tensor_add(outS[:], O2[:], bias_sb[:, 2 * CCH:3 * CCH])
    nc.sync.dma_start(out_d, outS[:])


_NC_CACHE = {}


def build_nc(reps=1):
    key = ("nc", reps)
    if key in _NC_CACHE:
        return _NC_CACHE[key]
    nc = bacc.Bacc(
        "TRN2", target_bir_lowering=False, debug=False,
        enable_asserts=False, num_devices=NCORES,
    )
    aps = {
        "xc": nc.dram_tensor("xc", [N, C], F32, kind="ExternalInput").ap(),
        "abf": nc.dram_tensor("abf", [2 * CG8 * P, C], FP8,
                              kind="ExternalInput").ap(),
        "wvp": nc.dram_tensor("wvp", [C, C], BF16, kind="ExternalInput").ap(),
        "wf1": nc.dram_tensor("wf1", [C, C], BF16, kind="ExternalInput").ap(),
        "wf2": nc.dram_tensor("wf2", [C, C], BF16, kind="ExternalInput").ap(),
        "biasR": nc.dram_tensor("biasR", [P, 3 * CCH], F32,
                                kind="ExternalInput").ap(),
        "ident": nc.dram_tensor("ident", [P, P], BF16,
                                kind="ExternalInput").ap(),
        "outT": nc.dram_tensor("outT", [P, CCH], F32,
                               kind="ExternalOutput").ap(),
    }
    with tile.TileContext(nc) as tc:
        for _ in range(reps):
            with ExitStack() as ctx:
                _build_body(ctx, tc, aps)
    nc.compile()
    _NC_CACHE[key] = nc
    return nc


def prep_in_maps(x_, Wq, Wk, Wv, Wp, bp, Wf1, bf1, Wf2, bf2):
    f32 = np.float32
    x_ = np.ascontiguousarray(np.asarray(x_, dtype=f32))
    A = (np.asarray(Wq, f32) @ np.asarray(Wk, f32).T) * np.float32(C ** -0.5)
    a_pad = np.zeros((2 * CG8 * P, C), f32)
    a_pad[:C] = A * np.float32(ASC)
    abf = np.ascontiguousarray(a_pad.astype(_F8))
    wvp = np.ascontiguousarray(
        (np.asarray(Wv, f32) @ np.asarray(Wp, f32)).astype(_BF))
    wf1 = np.ascontiguousarray(np.asarray(Wf1, f32).astype(_BF))
    wf2 = np.ascontiguousarray(np.asarray(Wf2, f32).astype(_BF))
    biasR = np.concatenate(
        [np.asarray(b, f32).reshape(CCH, P).T for b in (bp, bf1, bf2)], axis=1
    )
    biasR = np.ascontiguousarray(biasR)
    ident = np.eye(P, dtype=_BF)
    shared = {
        "abf": abf, "wvp": wvp, "wf1": wf1, "wf2": wf2,
        "biasR": biasR, "ident": ident,
    }
    return [dict(shared, xc=np.ascontiguousarray(x_[b])) for b in range(B)]


def assemble_output(results):
    out = np.empty((B, C), dtype=np.float32)
    for b in range(B):
        out[b] = np.asarray(results[b]["outT"], np.float32).T.reshape(C)
    return out


def kernel(**inputs) -> np.ndarray:
    nc = build_nc()
    in_maps = prep_in_maps(**inputs)
    res = run_bass_kernel_spmd(nc, in_maps, list(range(NCORES)))
    return assemble_output(res.results)


if __name__ == "__main__":
    import jax
    import reference as R
    inp = {k: np.asarray(v) for k, v in R.setup_inputs().items()}
    out = kernel(**inp)
    print(out.shape, out.dtype)
